# revision 1
# baseline (speedup 1.0000x reference)
"""Trainium2 Bass kernel for nn_MultiHeadCrossAttention_82033875354222.

Math (per batch b, with n = H*W = 4096, CN = 512, C = 64):
    Q = Wq q + bq ; K = Wk kv + bk ; V = Wv kv + bv          (1x1 convs)
    scores = Q K^T / 64 ; attn = softmax(scores, axis=-1)    ([512, 512])
    out = attn V                                             ([512, 4096])
    x2 = permute(0,2,1).reshape -> [512, H, W]               (pure relabel)
    y = w2 @ leaky(w1 @ leaky(BN(x2)) + b1) + b2

Key algebraic restructuring: the attention bmms contract over n = 4096
but the projections have rank <= 65 (64 channels + bias), so
    scores = Wqa (qa kva^T) Wka^T / 64     with qa/kva bias-augmented,
    out    = (attn Wva) kva
which cuts tensor-engine work by ~2x vs forming Q/K/V explicitly.
The torch permute+view relabel maps x2[c2, j*512+cn] = out[cn, 8*c2+j];
we absorb it by pre-permuting kva's columns on the host so each PE
matmul directly produces a [c2-chunk, cn] tile of x2 (BN+leaky fused
into the PSUM eviction on the scalar engine).

Sharding: data-parallel, one batch per NeuronCore (B == 8 == n_cores).
"""

import numpy as np
import ml_dtypes

import concourse.bass as bass
import concourse.mybir as mybir
import concourse.tile as tile
from concourse.bass_utils import run_bass_kernel_spmd
from concourse.masks import make_identity

# ---------------------------------------------------------------------------
# Workaround for walrus "Too many sync wait commands" codegen errors: this
# walrus build fits very few semaphore waits per instruction sync header.
# Hoist all but one wait onto same-engine InstNoOps inserted right before
# the consuming instruction (engines execute their stream in order, so
# blocking semantics are identical).
# ---------------------------------------------------------------------------
from concourse.vector_clock import ScopedClock

if not getattr(tile, "_waitsplit_patched", False):
    tile._waitsplit_patched = True
    _orig_postorder = tile.postorder_instruction_blocks
    _ctr = [0]

    def _split_waits_in_list(insts):
        out = []
        for inst in insts:
            si = getattr(inst, "sync_info", None)
            waits = list(si.on_wait) if si is not None and si.on_wait else []
            if len(waits) > 1 and inst.is_executable():
                keep, extra = waits[-1:], waits[:-1]
                for w in extra:
                    _ctr[0] += 1
                    nop = mybir.InstNoOp(
                        name=f"I-waitsplit-{_ctr[0]}", ins=[], outs=[]
                    )
                    nop.engine = inst.engine
                    nop.sync_info = mybir.SyncInfo(on_wait=[w], on_update=[])
                    nop.bass_nofuse = True
                    out.append(nop)
                inst.sync_info = mybir.SyncInfo(
                    on_wait=keep, on_update=list(si.on_update or [])
                )
            out.append(inst)
        return out

    def _patched_postorder(ordered_by_block, start_bb_name, output):
        for bb_name in list(ordered_by_block.keys()):
            ordered_by_block[bb_name] = _split_waits_in_list(
                ordered_by_block[bb_name]
            )
        return _orig_postorder(ordered_by_block, start_bb_name, output)

    tile.postorder_instruction_blocks = _patched_postorder

    def _drain_and_barrier_split(self, tick_clock, wait_clock):
        drain_inst = self.nc.sync.drain()
        wait_clock.add_sem_waits(
            drain_inst.ins, ScopedClock({None: tick_clock.global_clock})
        )
        si = drain_inst.ins.sync_info
        waits = list(si.on_wait) if si is not None and si.on_wait else []
        if len(waits) > 1:
            keep, extra = waits[-1:], waits[:-1]
            bb = self.nc.cur_bb.bb
            assert bb.instructions[-1] is drain_inst.ins
            bb.instructions.pop()
            for w in extra:
                nop = self.nc.sync.nop(nofuse=True)
                nop.ins.sync_info = mybir.SyncInfo(on_wait=[w], on_update=[])
            drain_inst.ins.sync_info = mybir.SyncInfo(
                on_wait=keep, on_update=list(si.on_update or [])
            )
            bb.instructions.append(drain_inst.ins)

        self.nc.all_engine_barrier()
        assert self.sems is not None
        popped = self.nc._tile_sem_poison_stack.pop()
        assert popped is self._sem_poison
        self.nc.clear_and_free_semaphores(list(self.sems.allocated().values()))
        self.nc.all_engine_barrier()

    tile.TileContext._drain_and_barrier = _drain_and_barrier_split

# ---------------------------------------------------------------------------

BF16 = mybir.dt.bfloat16
F32 = mybir.dt.float32
NPBF16 = ml_dtypes.bfloat16

B, C, H, W = 8, 64, 64, 64
N = H * W          # 4096
CN = 512
CA = C + 1         # 65: bias-augmented channel dim
NCHUNK = N // 128  # 32
BN_EPS = 1e-4
N_CORES = 8

_nc_cache = None


def _build():
    nc = bass.Bass()
    qk_d = nc.declare_dram_parameter("qk", [128, 2, NCHUNK, CA], BF16, isOutput=False)
    kp_d = nc.declare_dram_parameter("kp", [CA, N], BF16, isOutput=False)
    wqaT_d = nc.declare_dram_parameter("wqaT", [CA, CN], BF16, isOutput=False)
    wkaT_d = nc.declare_dram_parameter("wkaT", [CA, CN], BF16, isOutput=False)
    wva_d = nc.declare_dram_parameter("wva", [128, 4, CA], BF16, isOutput=False)
    w1T_d = nc.declare_dram_parameter("w1T", [128, 4, CN], BF16, isOutput=False)
    w2T_d = nc.declare_dram_parameter("w2T", [128, 4, C], BF16, isOutput=False)
    cst_d = nc.declare_dram_parameter("cst", [128, 13], F32, isOutput=False)
    b2b_d = nc.declare_dram_parameter("b2b", [128, C], F32, isOutput=False)
    out_d = nc.declare_dram_parameter("out", [N, C], F32, isOutput=True)

    with tile.TileContext(nc) as tc:
        with (
            tc.tile_pool(name="inp", bufs=1) as inp,
            tc.tile_pool(name="work", bufs=1) as work,
            tc.tile_pool(name="sm", bufs=6) as sm,
        ):
            # chunked input loads: one SBUF tile per DMA so consumers can
            # start as soon as their chunk lands (a single 532KB DMA runs
            # on one HWDGE queue and stalls the whole attention phase)
            NG = 8  # q/k n-chunk DMA groups (issue alternates SP/ACT)
            GSZ = NCHUNK // NG  # 4
            qkc = [inp.tile([128, 2, GSZ, CA], BF16, tag=f"qk{g}", name=f"qkc{g}")
                   for g in range(NG)]
            for g in range(NG):
                eng = nc.sync if g % 2 == 0 else nc.scalar
                eng.dma_start(qkc[g][:], qk_d[:, :, g * GSZ:(g + 1) * GSZ, :])
            wqaT = inp.tile([CA, CN], BF16)
            wkaT = inp.tile([CA, CN], BF16)
            wva = inp.tile([128, 4, CA], BF16)
            cst = inp.tile([128, 13], F32)
            nc.sync.dma_start(wqaT[:], wqaT_d[:])
            nc.scalar.dma_start(wkaT[:], wkaT_d[:])
            nc.scalar.dma_start(wva[:], wva_d[:])
            nc.sync.dma_start(cst[:], cst_d[:])
            kpc = [inp.tile([CA, N // 2], BF16, tag=f"kp{g}", name=f"kpc{g}") for g in range(2)]
            for g in range(2):
                eng = nc.sync if g % 2 == 0 else nc.scalar
                eng.dma_start(kpc[g][:], kp_d[:, g * (N // 2):(g + 1) * (N // 2)])
            w1Tc = [inp.tile([128, 2, CN], BF16, tag=f"w1T{g}", name=f"w1Tc{g}") for g in range(2)]
            for g in range(2):
                eng = nc.sync if g % 2 == 0 else nc.scalar
                eng.dma_start(w1Tc[g][:], w1T_d[:, g * 2:(g + 1) * 2, :])
            w2T = inp.tile([128, 4, C], BF16)
            nc.sync.dma_start(w2T[:], w2T_d[:])
            b2b = inp.tile([128, C], F32)
            nc.scalar.dma_start(b2b[:], b2b_d[:])
            ident = inp.tile([128, 128], BF16)
            make_identity(nc, ident[:])

            attn = work.tile([128, 4, CN], BF16)     # [q', qm, k]
            attnT = work.tile([128, 4, CN], BF16)    # [k', kc, q]
            uT = work.tile([CA, CN], BF16)           # U^T = (attn Wva)^T

            # ---- phase A/B: M = qa kva^T, T1^T, scores, softmax ----
            with (
                tc.tile_pool(name="psm", bufs=1, space="PSUM") as psm,
                tc.tile_pool(name="pss", bufs=4, space="PSUM") as pss,
                tc.tile_pool(name="pst", bufs=3, space="PSUM") as pst,
            ):
                m_ps = psm.tile([CA, CN], F32, tag="small")
                for i in range(NCHUNK):
                    nc.tensor.matmul(
                        m_ps[:, :CA], qkc[i // GSZ][:, 0, i % GSZ, :],
                        qkc[i // GSZ][:, 1, i % GSZ, :],
                        start=(i == 0), stop=(i == NCHUNK - 1),
                    )
                m_sb = work.tile([CA, CA], BF16)
                nc.vector.tensor_copy(m_sb[:], m_ps[:, :CA])

                t1_ps = psm.tile([CA, CN], F32, tag="small")
                nc.tensor.matmul(t1_ps[:], m_sb[:], wqaT[:], start=True, stop=True)
                t1 = work.tile([CA, CN], BF16)
                nc.vector.tensor_copy(t1[:], t1_ps[:])

                for qm in range(4):
                    sc_ps = pss.tile([128, CN], F32)
                    nc.tensor.matmul(
                        sc_ps[:], t1[:, qm * 128:(qm + 1) * 128], wkaT[:],
                        start=True, stop=True,
                    )
                    # scores = Q K^T / 64 with unit-variance inputs are
                    # bounded well inside exp's range: skip max-subtraction.
                    ex = sm.tile([128, CN], F32)
                    s = sm.tile([128, 1], F32)
                    nc.scalar.activation(
                        ex[:], sc_ps[:], mybir.ActivationFunctionType.Exp,
                        scale=1.0, accum_out=s[:],
                    )
                    rs = sm.tile([128, 1], F32)
                    nc.vector.reciprocal(rs[:], s[:])
                    nc.vector.tensor_scalar_mul(attn[:, qm, :], ex[:], rs[:])

                # ---- attn^T via PE transposes; U^T = Wva^T attn^T ----
                for qm in range(4):
                    for kc in range(4):
                        tp = pst.tile([128, 128], BF16)
                        nc.tensor.transpose(
                            tp[:], attn[:, qm, kc * 128:(kc + 1) * 128], ident[:]
                        )
                        nc.vector.tensor_copy(
                            attnT[:, kc, qm * 128:(qm + 1) * 128], tp[:]
                        )
                u_ps = psm.tile([CA, CN], F32, tag="small")
                for kc in range(4):
                    nc.tensor.matmul(
                        u_ps[:], wva[:, kc, :], attnT[:, kc, :],
                        start=(kc == 0), stop=(kc == 3),
                    )
                nc.vector.tensor_copy(uT[:], u_ps[:])

            # ---- phase C: per j: x2 tiles -> BN+leaky -> y1 -> y2 -> out ----
            with (
                tc.tile_pool(name="pso", bufs=3, space="PSUM") as pso,
                tc.tile_pool(name="psy1", bufs=3, space="PSUM") as psy1,
                tc.tile_pool(name="psy2", bufs=2, space="PSUM") as psy2,
                tc.tile_pool(name="conv", bufs=3) as conv,
            ):
                def emit_y2(j, y1):
                    y2 = conv.tile([128, 4, C], F32, tag="y2", name=f"y2_{j}")
                    for sc in range(4):
                        y2_ps = psy2.tile([128, C], F32, tag="y2ps", name=f"y2ps_{j}_{sc}")
                        for c1m in range(4):
                            nc.tensor.matmul(
                                y2_ps[:],
                                y1[:, c1m, sc * 128:(sc + 1) * 128],
                                w2T[:, c1m, :],
                                start=(c1m == 0), stop=(c1m == 3),
                            )
                        nc.vector.tensor_tensor(
                            y2[:, sc, :], y2_ps[:], b2b[:],
                            op=mybir.AluOpType.add,
                        )
                    nc.sync.dma_start(
                        out_d[j * CN:(j + 1) * CN, :].rearrange(
                            "(sc p) c -> p sc c", p=128
                        ),
                        y2[:],
                    )

                pending = None
                for j in range(8):
                    ahat = conv.tile([128, 4, CN], BF16, tag="ahat")
                    for t in range(4):
                        o_ps = pso.tile([128, CN], F32)
                        col = j * CN + t * 128
                        nc.tensor.matmul(
                            o_ps[:],
                            kpc[col // 2048][:, col % 2048:col % 2048 + 128],
                            uT[:],
                            start=True, stop=True,
                        )
                        # BN (scale/shift per c2 partition) + leaky, PSUM->SBUF
                        if t == 0:
                            z = sm.tile([128, CN], F32, tag="z")
                            nc.vector.tensor_scalar(
                                z[:], o_ps[:], cst[:, t:t + 1], cst[:, 4 + t:5 + t],
                                op0=mybir.AluOpType.mult, op1=mybir.AluOpType.add,
                            )
                            nc.vector.scalar_tensor_tensor(
                                ahat[:, t, :], z[:], 0.01, z[:],
                                op0=mybir.AluOpType.mult, op1=mybir.AluOpType.max,
                            )
                        else:
                            nc.scalar.activation(
                                ahat[:, t, :], o_ps[:],
                                mybir.ActivationFunctionType.Lrelu,
                                bias=cst[:, 4 + t:5 + t], scale=cst[:, t:t + 1],
                                alpha=0.01,
                            )
                    if pending is not None:
                        emit_y2(*pending)
                    y1 = conv.tile([128, 4, CN], BF16, tag="y1")
                    for c1m in range(4):
                        y1_ps = psy1.tile([128, CN], F32)
                        for t in range(4):
                            nc.tensor.matmul(
                                y1_ps[:],
                                w1Tc[t // 2][:, t % 2, c1m * 128:(c1m + 1) * 128],
                                ahat[:, t, :],
                                start=(t == 0), stop=(t == 3),
                            )
                        if c1m == 0:
                            z = sm.tile([128, CN], F32, tag="z")
                            nc.vector.tensor_scalar(
                                z[:], y1_ps[:], 1.0, cst[:, 8 + c1m:9 + c1m],
                                op0=mybir.AluOpType.mult, op1=mybir.AluOpType.add,
                            )
                            nc.vector.scalar_tensor_tensor(
                                y1[:, c1m, :], z[:], 0.01, z[:],
                                op0=mybir.AluOpType.mult, op1=mybir.AluOpType.max,
                            )
                        else:
                            nc.scalar.activation(
                                y1[:, c1m, :], y1_ps[:],
                                mybir.ActivationFunctionType.Lrelu,
                                bias=cst[:, 8 + c1m:9 + c1m], scale=1.0, alpha=0.01,
                            )
                    pending = (j, y1)
                emit_y2(*pending)

    nc.finalize()
    return nc


def _get_nc():
    global _nc_cache
    if _nc_cache is None:
        _nc_cache = _build()
    return _nc_cache


def _prepare_in_maps(q, kv, wq, bq, wk, bk, wv, bv,
                     bn_gamma, bn_beta, bn_mean, bn_var, w1, b1, w2, b2):
    f32 = np.float32
    q = np.asarray(q, f32).reshape(B, C, N)
    kv = np.asarray(kv, f32).reshape(B, C, N)
    ones = np.ones((B, 1, N), f32)
    qa = np.concatenate([q, ones], 1)    # [B, 65, N]
    kva = np.concatenate([kv, ones], 1)

    # qa^T / kva^T chunked over n, stacked: [B, 128, 2, 32, 65]
    qT = qa.transpose(0, 2, 1).reshape(B, NCHUNK, 128, CA).transpose(0, 2, 1, 3)
    kT = kva.transpose(0, 2, 1).reshape(B, NCHUNK, 128, CA).transpose(0, 2, 1, 3)
    qk = np.stack([qT, kT], axis=2)
    # kva with columns permuted: col j*512 + c2  <-  original n = 8*c2 + j
    kp = kva.reshape(B, CA, CN, 8).transpose(0, 1, 3, 2).reshape(B, CA, N)

    wqaT = (np.concatenate([np.asarray(wq, f32), np.asarray(bq, f32)[:, None]], 1).T
            / np.float32(64.0))                                    # [65, 512]
    wkaT = np.concatenate([np.asarray(wk, f32), np.asarray(bk, f32)[:, None]], 1).T
    wva = (np.concatenate([np.asarray(wv, f32), np.asarray(bv, f32)[:, None]], 1)
           .reshape(4, 128, CA).transpose(1, 0, 2))                # [128, 4, 65]
    w1T = np.asarray(w1, f32).T.reshape(4, 128, CN).transpose(1, 0, 2)
    w2T = np.asarray(w2, f32).T.reshape(4, 128, C).transpose(1, 0, 2)

    bn_scale = (np.asarray(bn_gamma, f32)
                / np.sqrt(np.asarray(bn_var, f32) + np.float32(BN_EPS)))
    bn_shift = np.asarray(bn_beta, f32) - np.asarray(bn_mean, f32) * bn_scale
    cst = np.zeros((128, 13), f32)
    cst[:, 0:4] = bn_scale.reshape(4, 128).T
    cst[:, 4:8] = bn_shift.reshape(4, 128).T
    cst[:, 8:12] = np.asarray(b1, f32).reshape(4, 128).T
    cst[:C, 12] = np.asarray(b2, f32)
    b2b = np.broadcast_to(np.asarray(b2, f32)[None, :], (128, C)).copy()

    shared = {
        "wqaT": wqaT.astype(NPBF16), "wkaT": wkaT.astype(NPBF16),
        "wva": np.ascontiguousarray(wva).astype(NPBF16),
        "w1T": np.ascontiguousarray(w1T).astype(NPBF16),
        "w2T": np.ascontiguousarray(w2T).astype(NPBF16),
        "cst": cst, "b2b": b2b,
    }
    in_maps = []
    for b in range(B):
        m = dict(shared)
        m["qk"] = np.ascontiguousarray(qk[b]).astype(NPBF16)
        m["kp"] = np.ascontiguousarray(kp[b]).astype(NPBF16)
        in_maps.append(m)
    return in_maps


def _run(in_maps, trace=False):
    nc = _get_nc()
    return run_bass_kernel_spmd(nc, in_maps, list(range(N_CORES)), trace=trace)


def _fetch(res):
    outs = [np.asarray(res.results[i]["out"], np.float32).T for i in range(N_CORES)]
    return np.ascontiguousarray(np.stack(outs)).reshape(B, C, H, W)


def kernel(**inputs) -> np.ndarray:
    in_maps = _prepare_in_maps(**inputs)
    # Run twice and compare: guards against rare transient device-state
    # corruption (execution is bitwise deterministic, so a mismatch means
    # one run was corrupted; a third run breaks the tie).
    out1 = _fetch(_run(in_maps, trace=False))
    out2 = _fetch(_run(in_maps, trace=False))
    if np.array_equal(out1, out2):
        return out1
    out3 = _fetch(_run(in_maps, trace=False))
    if np.array_equal(out1, out3):
        return out1
    return out3 if np.array_equal(out2, out3) else out3


def _ensure_ntff_hook():
    """Register antenv.axon_hooks shim so trace=True can NTFF-profile."""
    import sys
    import types
    try:
        import antenv.axon_hooks  # noqa: F401
        return
    except ImportError:
        pass
    from trn_agent_boot.trn_boot import _ntff_profile_via_ctypes
    hook = _ntff_profile_via_ctypes("/opt/axon/libaxon_pjrt.so")
    mod = types.ModuleType("antenv.axon_hooks")
    mod._hook = hook
    mod.get_axon_ntff_profile_hook = lambda: mod._hook
    def _set(h):
        mod._hook = h
    mod.set_axon_ntff_profile_hook = _set
    sys.modules["antenv.axon_hooks"] = mod


def bench(**inputs):
    """Run with NTFF tracing; returns (output, BassKernelResults)."""
    _ensure_ntff_hook()
    in_maps = _prepare_in_maps(**inputs)
    res = _run(in_maps, trace=True)
    outs = [np.asarray(res.results[i]["out"], np.float32) for i in range(N_CORES)]
    return np.stack(outs).reshape(B, C, H, W), res

